# revision 57
# baseline (speedup 1.0000x reference)
"""MultiHeadAttention (B=2, S=2048, D=1024, H=16, causal) on 8 trn2 NeuronCores.

Sharding: tensor-parallel over heads (2 heads/core) for QKV projections and
attention; four AllToAlls (one per (batch, seq-half) segment) re-shard context
rows so the output projection is data-parallel over rows; bias added on
device. Host only slices/transposes/casts inputs and reassembles outputs.

Per-core output rows: global rows [c*256,(c+1)*256) (batch 0 part) and
[2048+c*256, 2048+(c+1)*256) (batch 1 part).

Schedule notes (v3):
  - x is loaded in 1024-row ranges (range 0 split into fine pieces across
    all three DMA-capable queues) so projections start ~12us in; V is
    projected directly in [row, dim] layout (x-slice stationary) so no
    transposes are needed for the attention V operand.
  - attention / norm / AllToAll / output projection are interleaved per
    segment so the middle collectives hide under attention compute and
    the PE never sees a multi-us gap (HAM duty stays high).
  - softmax denominators ride as a ones-column in the AV matmul (row 64
    of the [65, QT] PSUM tile); reciprocal_approx_fast runs over the
    whole base-0 cst tile (junk rows unread), a K=1 ones matmul
    broadcasts the reciprocal row, one DVE multiply normalizes. No DMA
    and no scalar op in this chain: DMAs here share pooled completion
    semaphores with the collective-gated a2a_sb loads and serialize
    attention behind collectives when a peer core launches late.
  - all a2a_sb loads sit at the tail behind a WAW gate copy so the Tile
    scheduler cannot hoist them (and their semaphore baselines) into the
    mid-pipeline; outproj matmuls for segments 0-2 run as real filler
    inside the final AllToAll's rendezvous window.
  - exec time = barrier skew (15-95us, environmental) + ~155us pipeline.
  - reference quirk preserved: scale = 1/(D**0.25).
"""

import os
import sys
import types

import numpy as np
import ml_dtypes

N_CORES = 8
B, S, D = 2, 2048, 1024
H = 16
HEAD = 64
ROWS = B * S               # 4096
ROWS_PER_CORE = ROWS // N_CORES  # 512
INV_SCALE = 1.0 / (D ** 0.25)
QT = 512                   # query tile (free dim)
KT = 128                   # key tile (partition dim)
RH = 128                   # rows per core per A2A segment

BF16 = ml_dtypes.bfloat16
FP8 = ml_dtypes.float8_e4m3

_compiled = None


def _install_axon_profile_shim():
    """Provide antenv.axon_hooks (missing from this image) so trace=True works,
    and neuter the artifact upload (no bucket access in-container)."""
    if "antenv.axon_hooks" not in sys.modules:
        mod = types.ModuleType("antenv.axon_hooks")
        mod._hook = None
        mod.set_axon_ntff_profile_hook = lambda h: setattr(mod, "_hook", h)
        mod.get_axon_ntff_profile_hook = lambda: mod._hook
        sys.modules["antenv.axon_hooks"] = mod
        try:
            import antenv
            antenv.axon_hooks = mod
        except ImportError:
            pass
    mod = sys.modules["antenv.axon_hooks"]
    if mod._hook is None:
        try:
            from trn_agent_boot.trn_boot import _ntff_profile_via_ctypes
            mod.set_axon_ntff_profile_hook(
                _ntff_profile_via_ctypes("/opt/axon/libaxon_pjrt.so"))
        except Exception:
            pass
    try:
        import concourse.bass_utils as bu
        bu.upload_artifacts = lambda tmpdir: tmpdir
    except Exception:
        pass


def _build_program():
    import concourse.bass as bass
    import concourse.bacc as bacc
    import concourse.mybir as mybir
    import concourse.tile as tile
    from concourse.bass import ts

    f32 = mybir.dt.float32
    f32r = mybir.dt.float32r
    bf16 = mybir.dt.bfloat16
    fp8 = mybir.dt.float8e4
    Exp = mybir.ActivationFunctionType.Exp

    nc = bacc.Bacc(num_devices=N_CORES)

    xT = nc.dram_tensor("xT", [D, ROWS], bf16, kind="ExternalInput")
    wqT = nc.dram_tensor("wqT", [128, 8, 128], bf16, kind="ExternalInput")
    wkT = nc.dram_tensor("wkT", [128, 8, 128], bf16, kind="ExternalInput")
    wvT = nc.dram_tensor("wvT", [128, 8, 128], bf16, kind="ExternalInput")
    woT = nc.dram_tensor("woT", [128, 8, D], bf16, kind="ExternalInput")
    bo = nc.dram_tensor("bo", [D], f32, kind="ExternalInput")
    masksq = nc.dram_tensor("masksq", [128, 128], bf16, kind="ExternalInput")
    sel = nc.dram_tensor("sel", [4, 4 * HEAD], bf16, kind="ExternalInput")
    out_rows = nc.dram_tensor("out_rows", [ROWS_PER_CORE, D], f32,
                              kind="ExternalOutput")
    debug = bool(os.environ.get("K_DEBUG"))
    if debug:
        dbg_q = nc.dram_tensor("dbg_q", [128, ROWS], bf16,
                               kind="ExternalOutput")
        dbg_k = nc.dram_tensor("dbg_k", [128, ROWS], bf16,
                               kind="ExternalOutput")
        dbg_v = nc.dram_tensor("dbg_v", [128, 8, 4, 2, HEAD + 1], bf16,
                               kind="ExternalOutput")

    with tile.TileContext(nc) as tc:
        with (
            tc.tile_pool(name="persist", bufs=1) as persist,
            tc.tile_pool(name="cp", bufs=4) as cp,
            tc.tile_pool(name="attn", bufs=6) as attn_pool,
            tc.tile_pool(name="ps_work", bufs=3, space="PSUM") as ps_work,
            tc.tile_pool(name="ps_scores", bufs=2, space="PSUM") as ps_scores,
            tc.tile_pool(name="ps_bc", bufs=1, space="PSUM") as ps_bc,
            tc.tile_pool(name="dram", bufs=1, space="DRAM") as dram,
        ):
            # ---- persistent SBUF state ----
            xT_sb = persist.tile([128, 8, ROWS], bf16)        # 64 KB/part
            wq_sb = persist.tile([128, 8, 128], bf16)
            wk_sb = persist.tile([128, 8, 128], bf16)
            wv_sb = persist.tile([128, 8, 128], bf16)
            woT_sb = persist.tile([128, 8, D], bf16)          # 16 KB/part
            qT_sb = persist.tile([128, ROWS], bf16)           # 8 KB/part
            kT_sb = persist.tile([128, ROWS], bf16)
            # v in [key-row, head, dim] layout, groups of 4 key-row tiles;
            # col HEAD is the ones column for the softmax denominator: it
            # lands on PSUM partition 64 (a legal AP base) where the DVE
            # reciprocal reads it directly -- no DMA gather (den DMAs
            # share pooled completion semaphores with the collective-gated
            # a2a_sb loads and stall the scalar queue when a peer is late)
            v2g = [persist.tile([128, 4, 2, HEAD + 1], bf16, tag=f"v2g{g}",
                                name=f"v2g{g}") for g in range(8)]
            ctx_sb = [persist.tile([64, ROWS], bf16, tag=f"ctx{h}",
                                   name=f"ctx{h}")
                      for h in range(2)]
            mask_sb = persist.tile([128, 128], bf16)
            sel_sb = persist.tile([4, 4 * HEAD], bf16)
            # row 64 = 1.0: stationary for the den-broadcast matmul,
            # partition-aligned with the denominator row (base 64)
            ones65_sb = persist.tile([65, HEAD], bf16)
            bo_sb = persist.tile([128, D], f32)
            a2a_sb = [persist.tile([128, 8, RH], bf16, tag=f"a2a{g}",
                                   name=f"a2a{g}") for g in range(4)]

            warm_sb = persist.tile([128, 512], bf16)

            # ---- HAM warmup: DVE memsets a junk tile at t=0, then a short
            #      burst of matmuls keeps the PE activity window busy so the
            #      clock gate opens (1.2 -> 2.4 GHz) before real work ----
            # enough junk to keep the PE continuously busy until the first
            # x pieces land (~12us): an idle PE drops the HAM duty to 4/8
            # right when the projections start, doubling their time
            nc.vector.memset(warm_sb[:], 0.0)
            for wi in range(0 if os.environ.get("K_NOWARM") else 16):
                ps_w = ps_work.tile([128, 512], f32, tag="work",
                                    name=f"warm{wi}")
                nc.tensor.matmul(ps_w, warm_sb[:, 0:128], warm_sb[:],
                                 start=True, stop=True)

            # ---- small loads: weights on gpsimd (wq first: it gates the
            #      very first projection matmul); tiny constants on scalar.
            #      The scalar queue carries NO bulk DMA so the engine is
            #      always free for the softmax Exp activations. ----
            nc.gpsimd.dma_start(wq_sb[:], wqT[:])
            nc.gpsimd.dma_start(wv_sb[:], wvT[:])
            nc.gpsimd.dma_start(wk_sb[:], wkT[:])
            nc.scalar.dma_start(mask_sb[:], masksq[:])
            nc.scalar.dma_start(sel_sb[:], sel[:])
            nc.scalar.dma_start(
                bo_sb[:], bass.AP(tensor=bo, offset=0,
                                  ap=[[0, 128], [1, D]]))
            nc.gpsimd.memset(ones65_sb[HEAD:HEAD + 1, :], 1.0)
            for g8 in range(8):
                nc.gpsimd.memset(v2g[g8][:, :, :, HEAD:HEAD + 1], 1.0)

            # ---- x loads. Range 0 gates the whole pipeline: its 16 fine
            #      pieces go first, byte-balanced per queue (gpsimd already
            #      carries 0.77 MB of weights so it gets one piece; sync 7,
            #      scalar 8) so all pieces land together ~19us in.
            #      Piece-major order: the cols-0:511 half that the first
            #      projections touch lands first. Ranges 1-3 follow on
            #      sync+gpsimd, 2:1 toward sync to offset woT (2 MB) on
            #      gpsimd; scalar stays free for the softmax Exps. ----
            pi = 0
            for piece in range(2):
                for kt in range(8):
                    if pi == 14:
                        eng = nc.gpsimd
                    else:
                        eng = nc.sync if pi % 2 == 0 else nc.scalar
                    c0 = piece * 512
                    eng.dma_start(xT_sb[:, kt, c0:c0 + 512],
                                  xT[ts(kt, 128), c0:c0 + 512])
                    pi += 1
            pi = 0
            for rr in range(1, 4):
                for kt in range(8):
                    eng = (nc.sync, nc.gpsimd)[pi % 2]
                    eng.dma_start(xT_sb[:, kt, ts(rr, 1024)],
                                  xT[ts(kt, 128), ts(rr, 1024)])
                    pi += 1
            # woT is not needed until the first output projection (~60us in);
            # keep it behind the x chunks so it can't stall the projections
            nc.gpsimd.dma_start(woT_sb[:], woT[:])

            def proj_range(rr, vbs=(0, 1)):
                """QKV projections for global rows [rr*1024, (rr+1)*1024).

                V is produced directly in [row, dim] layout (x-rows slice as
                the stationary operand, N=128). Each contraction step runs
                q, v, v, k, v, v so the two 128-column V LDWEIGHTS hide
                under the 512-wide Q/K streams."""
                for vb in vbs:
                    g8 = rr * 2 + vb
                    rt = 2 * rr + vb
                    pv = ps_work.tile([128, 4, 128], f32, tag="work",
                                      name=f"pv{g8}")
                    pq = ps_work.tile([128, 512], f32, tag="work",
                                      name=f"pq{rt}")
                    pk = ps_work.tile([128, 512], f32, tag="work",
                                      name=f"pk{rt}")
                    # V accumulation groups run one s-subtile at a time
                    # (concurrently-open groups in one PSUM bank corrupt
                    # each other); Q/K streams live in other banks.
                    vi = 0
                    for kt in range(8):
                        se = dict(start=(kt == 0), stop=(kt == 7))
                        nc.tensor.matmul(pq, wq_sb[:, kt, :],
                                         xT_sb[:, kt, ts(rt, 512)], **se)
                        for _ in range(2):
                            s, vkt = vi // 8, vi % 8
                            r0 = (g8 * 4 + s) * 128
                            nc.tensor.matmul(pv[:, s, :],
                                             xT_sb[:, vkt, r0:r0 + 128],
                                             wv_sb[:, vkt, :],
                                             start=(vkt == 0), stop=(vkt == 7))
                            vi += 1
                        nc.tensor.matmul(pk, wk_sb[:, kt, :],
                                         xT_sb[:, kt, ts(rt, 512)], **se)
                        for _ in range(2):
                            s, vkt = vi // 8, vi % 8
                            r0 = (g8 * 4 + s) * 128
                            nc.tensor.matmul(pv[:, s, :],
                                             xT_sb[:, vkt, r0:r0 + 128],
                                             wv_sb[:, vkt, :],
                                             start=(vkt == 0), stop=(vkt == 7))
                            vi += 1
                    nc.vector.tensor_copy(
                        v2g[g8][:, :, :, 0:HEAD],
                        pv.rearrange("p s (h d) -> p s h d", h=2))
                    nc.vector.tensor_copy(qT_sb[:, ts(rt, 512)], pq)
                    nc.vector.tensor_copy(kT_sb[:, ts(rt, 512)], pk)

            def attention_qt(b, qt, after_jk1=None):
                """Attention for one query tile (512 rows).

                Softmax normalization is fused per (qt, head): reciprocal of
                the PSUM denominator row, a ones-column matmul broadcasts it
                over the 64 ctx partitions, and one DVE multiply writes the
                normalized ctx straight from PSUM to SBUF. `after_jk1` is
                issued once two key blocks are in flight — the slot where a
                previous tile's norm can run without stalling on its
                reciprocal chain."""
                q0 = b * S + qt * QT
                n_k = 4 * qt + 4
                ps_av = [ps_work.tile([HEAD + 1, QT], f32, tag="work",
                                      name=f"av{b}_{qt}_{h}")
                         for h in range(2)]
                for jk in range(n_k):
                    o = jk - 4 * qt       # >=0 on the diagonal band
                    c0 = max(o, 0) * 128  # first live query column
                    k0 = b * S + jk * KT
                    ps_s = ps_scores.tile([128, 2, QT], f32, tag="sc",
                                          name=f"sc{b}_{qt}_{jk}")
                    at = attn_pool.tile([128, 2, QT], bf16,
                                        tag=f"at{jk % 2}", bufs=4,
                                        name=f"at{b}_{qt}_{jk}")
                    for h in range(2):
                        hs = slice(h * HEAD, (h + 1) * HEAD)
                        nc.tensor.matmul(
                            ps_s[:, h, c0:QT],
                            kT_sb[hs, k0:k0 + KT],
                            qT_sb[hs, q0 + c0:q0 + QT],
                            start=True, stop=True)
                    if jk < 2:
                        # pipe fill: per-head Exp halves so the first AV
                        # matmul starts ~0.5us earlier at each qt boundary
                        for h in range(2):
                            nc.scalar.activation(at[:, h, c0:QT],
                                                 ps_s[:, h, c0:QT],
                                                 Exp, scale=INV_SCALE)
                    else:
                        nc.scalar.activation(at[:, :, c0:QT],
                                             ps_s[:, :, c0:QT],
                                             Exp, scale=INV_SCALE)
                    if o >= 0:
                        # partial causal sub-block: cols [c0, c0+128)
                        nc.vector.tensor_mul(
                            at[:, :, c0:c0 + 128],
                            at[:, :, c0:c0 + 128],
                            mask_sb[:, None, :].to_broadcast([128, 2, 128]))
                    rt128 = b * 16 + jk
                    for h in range(2):
                        nc.tensor.matmul(
                            ps_av[h][:, c0:QT],
                            v2g[rt128 // 4][:, rt128 % 4, h, :],
                            at[:, h, c0:QT],
                            start=(jk == 0), stop=(jk == n_k - 1))
                    if jk == 1 and after_jk1 is not None:
                        after_jk1()
                gq = b * 4 + qt
                csts = []
                recbs = []
                for h in range(2):
                    # one copy moves ctx AND the denominator row (PSUM
                    # partition 64, the ones column) out of PSUM
                    cst = attn_pool.tile([HEAD + 1, QT], f32, tag="cst",
                                         bufs=4, name=f"cst{b}_{qt}_{h}")
                    nc.vector.tensor_copy(cst, ps_av[h][0:HEAD + 1, :])
                    # reciprocal issues eagerly on DVE so its latency
                    # hides under attention. The custom approx op (~18
                    # bits, single DVE pass, 5x cheaper) works on
                    # partition-0-based APs.
                    # reciprocal on the SCALAR engine (activation LUT),
                    # lane-aligned on partition 64: no DMA in the den
                    # chain, so no pooled-DMA-semaphore entanglement with
                    # the collective-gated a2a_sb loads
                    # (reciprocal_approx_fast is partition-0-only and the
                    # plain DVE InstReciprocal costs ~4us)
                    recb = attn_pool.tile([65, QT], bf16, tag="recb",
                                          bufs=4, name=f"recb{gq}_{h}")
                    recf = attn_pool.tile([65, QT], f32, tag="recf",
                                          bufs=4, name=f"recf{gq}_{h}")
                    # the approx op needs a base-0 AP: run it over the
                    # whole [65, QT] tile -- rows 0..63 produce junk
                    # reciprocals of ctx values that are never read; row
                    # 64 is the real denominator
                    nc.vector.reciprocal_approx_fast(recf[:], cst[:])
                    nc.vector.tensor_copy(recb[HEAD:HEAD + 1, :],
                                          recf[HEAD:HEAD + 1, :])
                    csts.append(cst)
                    recbs.append(recb)

                def norm_thunk(gq=gq, q0=q0, csts=csts, recbs=recbs):
                    # PE-side broadcast of the reciprocal row + one DVE
                    # multiply, issued later so the in-order PE stream
                    # never waits on the den chain
                    for h in range(2):
                        ps_b = ps_bc.tile([HEAD, QT], f32, tag="bc",
                                          name=f"bc{gq}_{h}")
                        nc.tensor.matmul(ps_b,
                                         ones65_sb[HEAD:HEAD + 1, :],
                                         recbs[h][HEAD:HEAD + 1, :],
                                         start=True, stop=True)
                        nc.vector.tensor_mul(
                            ctx_sb[h][:, q0:q0 + QT],
                            csts[h][0:HEAD, :],
                            ps_b[:])
                return norm_thunk

            def attention_half(b, half):
                return [attention_qt(b, 2 * half),
                        attention_qt(b, 2 * half + 1)]

            a2a_ins = [None] * 4

            def a2a_stage(b, half, qh):
                """Stage one 512-row query tile's ctx into the segment's
                A2A input buffer (issued right after that tile's norm so
                the copy hides under the next tile's attention)."""
                g = b * 2 + half
                r0 = b * S + half * (S // 2)
                if a2a_ins[g] is None:
                    a2a_ins[g] = dram.tile([8, 128, RH], bf16,
                                           tag=f"a2ain{g}", name=f"a2ain{g}")
                for h in range(2):
                    nc.sync.dma_start(
                        a2a_ins[g][qh * 4:(qh + 1) * 4,
                                   h * 64:(h + 1) * 64, :]
                        .rearrange("s p r -> p s r"),
                        ctx_sb[h][:, r0 + qh * QT:r0 + (qh + 1) * QT]
                        .rearrange("p (s r) -> p s r", s=4))

            def a2a_seg(b, half, staged=()):
                import concourse.mybir as mybir
                g = b * 2 + half
                # A2A for segment g: shard s = rows [b*2048+half*1024+s*128,+128)
                for qh in range(2):
                    if qh not in staged:
                        a2a_stage(b, half, qh)
                a2a_out = dram.tile([8, 128, RH], bf16, tag=f"a2aout{g}",
                                    name=f"a2aout{g}")
                nc.gpsimd.collective_compute(
                    "AllToAll", mybir.AluOpType.bypass,
                    replica_groups=[list(range(N_CORES))],
                    ins=[a2a_ins[g][:].opt()], outs=[a2a_out[:].opt()])
                a2a_outs[g] = a2a_out

            a2a_outs = [None] * 4

            def load_a2a(g):
                # a2a_sb load on the sync queue: the gpsimd queue head must
                # stay free for the next collective trigger
                for tb in range(2):
                    nc.sync.dma_start(
                        a2a_sb[g][:, 4 * tb:4 * tb + 4, :],
                        a2a_outs[g][4 * tb:4 * tb + 4]
                        .rearrange("t p r -> p t r"))

            def outproj_mm(g):
                # segment g rows land in out_rows[g*128:(g+1)*128]
                for nh in range(2):
                    ps = ps_work.tile([128, 512], f32, tag="work",
                                      name=f"po{g}_{nh}")
                    for t in range(8):
                        nc.tensor.matmul(ps,
                                         a2a_sb[g][:, t, :],
                                         woT_sb[:, t, ts(nh, 512)],
                                         start=(t == 0), stop=(t == 7))
                    ob = cp.tile([128, 512], f32, tag="ob", name=f"ob{g}_{nh}")
                    nc.vector.tensor_add(ob, ps, bo_sb[:, ts(nh, 512)])
                    nc.sync.dma_start(
                        out_rows[ts(g, 128), ts(nh, 512)], ob)

            def outproj(g):
                load_a2a(g)
                outproj_mm(g)

            # ---- interleaved schedule ----
            proj_range(0)
            if debug:
                nc.sync.dma_start(dbg_q[:], qT_sb[:])
                nc.sync.dma_start(dbg_k[:], kT_sb[:])
                for g8 in range(8):
                    nc.sync.dma_start(dbg_v[:, g8], v2g[g8][:])
            nt = attention_half(0, 0)
            # half of proj_range(1) runs between the attention and its
            # norms so the reciprocal chain (den DMA -> DVE recip -> bf16
            # copy) finishes under real PE work; then norms + the first A2A
            # trigger go BEFORE the rest of proj_range(1) so every core
            # stages its segment-0 ctx early and the first collective
            # (gated by the slowest core) completes well before outproj(0)
            proj_range(1, (0,))
            for t in nt:
                t()
            a2a_seg(0, 0)
            proj_range(1, (1,))
            nt = attention_half(0, 1)
            proj_range(2)
            for t in nt:
                t()
            a2a_seg(0, 1)
            nt = attention_half(1, 0)
            proj_range(3)
            for t in nt:
                t()
            a2a_seg(1, 0)
            t12 = attention_qt(1, 2)

            def norm_and_stage_12():
                t12()
                a2a_stage(1, 1, 0)  # qt=2 ctx ships under qt=3 attention

            t13 = attention_qt(1, 3, after_jk1=norm_and_stage_12)
            t13()
            a2a_seg(1, 1, staged=(0,))
            # tail: ALL a2a_sb loads live here, after the last norm — a
            # collective-gated DMA sequenced mid-pipeline (the Tile
            # scheduler hoists ready instructions regardless of issue
            # order) blocks the sync queue head AND, via pooled
            # DMA-completion semaphore baselines, the scalar den gathers,
            # serializing attention behind collectives when a peer core is
            # late. The tiny gate copies pin the loads behind the last
            # ctx write via a WAW dependency the scheduler must honor.
            for g in range(3):
                nc.gpsimd.tensor_copy(a2a_sb[g][0:1, 0:1, 0:1],
                                      ctx_sb[0][0:1, ROWS - 1:ROWS])
            load_a2a(0)
            load_a2a(1)
            load_a2a(2)
            for wi in range(48):
                ps_w = ps_scores.tile([128, 2, QT], f32, tag="sc",
                                      name=f"tailwarm{wi}")
                nc.tensor.matmul(ps_w[:, 0, :], warm_sb[:, 0:128], warm_sb[:],
                                 start=True, stop=True)
            outproj_mm(0)
            outproj_mm(1)
            outproj_mm(2)
            outproj(3)

    nc.finalize()  # Bacc.compile(): official wait-splitting & codegen passes
    return nc


def _make_masksq():
    p = np.arange(128)[:, None]
    r = np.arange(128)[None, :]
    return (p <= r).astype(BF16)


def _make_sel():
    # sel[k, u*64+m] = 1 if k == u : broadcasts den lane u over 64 partitions
    s = np.zeros((4, 4 * HEAD), np.float32)
    for u in range(4):
        s[u, u * HEAD:(u + 1) * HEAD] = 1.0
    return s.astype(BF16)


def _wlayout(wT):
    # [1024, m] -> [128, 8, m] with dst[p, t, :] = wT[t*128+p, :]
    m = wT.shape[1]
    return np.ascontiguousarray(
        wT.reshape(8, 128, m).transpose(1, 0, 2)).astype(BF16)


def _wlayout_dr8(wT):
    # [1024, m] -> fp8 [128, 4, 2, m]: dst[p, tp, j] = wT[tp*256+j*128+p, :]
    m = wT.shape[1]
    return np.ascontiguousarray(
        wT.reshape(4, 2, 128, m).transpose(2, 0, 1, 3)).astype(FP8)


def _shard_inputs(x, Wq, Wk, Wv, Wo, bo):
    xT = np.ascontiguousarray(
        x.reshape(ROWS, D).T).astype(BF16)            # [D, 4096]
    woT = _wlayout(Wo.T)                              # [128, 8, D]
    masksq = _make_masksq()
    sel = _make_sel()
    bo32 = np.ascontiguousarray(bo.astype(np.float32))
    maps = []
    for c in range(N_CORES):
        rs = slice(c * 128, (c + 1) * 128)
        maps.append({
            "xT": xT,
            "wqT": _wlayout(Wq[rs].T),
            "wkT": _wlayout(Wk[rs].T),
            "wvT": _wlayout(Wv[rs].T),
            "woT": woT,
            "bo": bo32,
            "masksq": masksq,
            "sel": sel,
        })
    return maps


def kernel(x, Wq, Wk, Wv, Wo, bo, trace=False):
    global _compiled
    _install_axon_profile_shim()
    from concourse.bass_utils import run_bass_kernel_spmd

    x = np.asarray(x, dtype=np.float32)
    Wq = np.asarray(Wq, dtype=np.float32)
    Wk = np.asarray(Wk, dtype=np.float32)
    Wv = np.asarray(Wv, dtype=np.float32)
    Wo = np.asarray(Wo, dtype=np.float32)
    bo = np.asarray(bo, dtype=np.float32)

    if _compiled is None:
        _compiled = _build_program()
    nc = _compiled

    in_maps = _shard_inputs(x, Wq, Wk, Wv, Wo, bo)
    res = run_bass_kernel_spmd(nc, in_maps, core_ids=list(range(N_CORES)),
                               trace=trace)
    out = np.empty((ROWS, D), np.float32)
    for c in range(N_CORES):
        r = res.results[c]["out_rows"]
        for g in range(4):
            b, half = g // 2, g % 2
            r0 = b * S + half * (S // 2) + c * RH
            out[r0:r0 + RH] = r[g * RH:(g + 1) * RH]
    out = out.reshape(B, S, D)
    if trace:
        kernel.last_exec_time_ns = res.exec_time_ns
        kernel.last_results = res
    return out



# revision 58
# speedup vs baseline: 1.0402x; 1.0402x over previous
"""MultiHeadAttention (B=2, S=2048, D=1024, H=16, causal) on 8 trn2 NeuronCores.

Sharding: tensor-parallel over heads (2 heads/core) for QKV projections and
attention; four AllToAlls (one per (batch, seq-half) segment) re-shard context
rows so the output projection is data-parallel over rows; bias added on
device. Host only slices/transposes/casts inputs and reassembles outputs.

Per-core output rows: global rows [c*256,(c+1)*256) (batch 0 part) and
[2048+c*256, 2048+(c+1)*256) (batch 1 part).

Schedule notes (v3):
  - x is loaded in 1024-row ranges (range 0 split into fine pieces across
    all three DMA-capable queues) so projections start ~12us in; V is
    projected directly in [row, dim] layout (x-slice stationary) so no
    transposes are needed for the attention V operand.
  - attention / norm / AllToAll / output projection are interleaved per
    segment so the middle collectives hide under attention compute and
    the PE never sees a multi-us gap (HAM duty stays high).
  - softmax denominators ride as a ones-column in the AV matmul (row 64
    of the [65, QT] PSUM tile); reciprocal_approx_fast runs over the
    whole base-0 cst tile (junk rows unread), a K=1 ones matmul
    broadcasts the reciprocal row, one DVE multiply normalizes. No DMA
    and no scalar op in this chain: DMAs here share pooled completion
    semaphores with the collective-gated a2a_sb loads and serialize
    attention behind collectives when a peer core launches late.
  - all a2a_sb loads sit at the tail behind a WAW gate copy so the Tile
    scheduler cannot hoist them (and their semaphore baselines) into the
    mid-pipeline; outproj matmuls for segments 0-2 run as real filler
    inside the final AllToAll's rendezvous window.
  - exec time = barrier skew (15-95us, environmental) + ~155us pipeline.
  - reference quirk preserved: scale = 1/(D**0.25).
"""

import os
import sys
import types

import numpy as np
import ml_dtypes

N_CORES = 8
B, S, D = 2, 2048, 1024
H = 16
HEAD = 64
ROWS = B * S               # 4096
ROWS_PER_CORE = ROWS // N_CORES  # 512
INV_SCALE = 1.0 / (D ** 0.25)
QT = 512                   # query tile (free dim)
KT = 128                   # key tile (partition dim)
RH = 128                   # rows per core per A2A segment

BF16 = ml_dtypes.bfloat16
FP8 = ml_dtypes.float8_e4m3

_compiled = None


def _install_axon_profile_shim():
    """Provide antenv.axon_hooks (missing from this image) so trace=True works,
    and neuter the artifact upload (no bucket access in-container)."""
    if "antenv.axon_hooks" not in sys.modules:
        mod = types.ModuleType("antenv.axon_hooks")
        mod._hook = None
        mod.set_axon_ntff_profile_hook = lambda h: setattr(mod, "_hook", h)
        mod.get_axon_ntff_profile_hook = lambda: mod._hook
        sys.modules["antenv.axon_hooks"] = mod
        try:
            import antenv
            antenv.axon_hooks = mod
        except ImportError:
            pass
    mod = sys.modules["antenv.axon_hooks"]
    if mod._hook is None:
        try:
            from trn_agent_boot.trn_boot import _ntff_profile_via_ctypes
            mod.set_axon_ntff_profile_hook(
                _ntff_profile_via_ctypes("/opt/axon/libaxon_pjrt.so"))
        except Exception:
            pass
    try:
        import concourse.bass_utils as bu
        bu.upload_artifacts = lambda tmpdir: tmpdir
    except Exception:
        pass


def _build_program():
    import concourse.bass as bass
    import concourse.bacc as bacc
    import concourse.mybir as mybir
    import concourse.tile as tile
    from concourse.bass import ts

    f32 = mybir.dt.float32
    f32r = mybir.dt.float32r
    bf16 = mybir.dt.bfloat16
    fp8 = mybir.dt.float8e4
    Exp = mybir.ActivationFunctionType.Exp

    nc = bacc.Bacc(num_devices=N_CORES)

    xT = nc.dram_tensor("xT", [D, ROWS], bf16, kind="ExternalInput")
    wqT = nc.dram_tensor("wqT", [128, 8, 128], bf16, kind="ExternalInput")
    wkT = nc.dram_tensor("wkT", [128, 8, 128], bf16, kind="ExternalInput")
    wvT = nc.dram_tensor("wvT", [128, 8, 128], bf16, kind="ExternalInput")
    woT = nc.dram_tensor("woT", [128, 8, D], bf16, kind="ExternalInput")
    bo = nc.dram_tensor("bo", [D], f32, kind="ExternalInput")
    masksq = nc.dram_tensor("masksq", [128, 128], bf16, kind="ExternalInput")
    sel = nc.dram_tensor("sel", [4, 4 * HEAD], bf16, kind="ExternalInput")
    out_rows = nc.dram_tensor("out_rows", [ROWS_PER_CORE, D], f32,
                              kind="ExternalOutput")
    debug = bool(os.environ.get("K_DEBUG"))
    if debug:
        dbg_q = nc.dram_tensor("dbg_q", [128, ROWS], bf16,
                               kind="ExternalOutput")
        dbg_k = nc.dram_tensor("dbg_k", [128, ROWS], bf16,
                               kind="ExternalOutput")
        dbg_v = nc.dram_tensor("dbg_v", [128, 8, 4, 2, HEAD + 1], bf16,
                               kind="ExternalOutput")

    with tile.TileContext(nc) as tc:
        with (
            tc.tile_pool(name="persist", bufs=1) as persist,
            tc.tile_pool(name="cp", bufs=4) as cp,
            tc.tile_pool(name="attn", bufs=6) as attn_pool,
            tc.tile_pool(name="ps_work", bufs=3, space="PSUM") as ps_work,
            tc.tile_pool(name="ps_scores", bufs=2, space="PSUM") as ps_scores,
            tc.tile_pool(name="ps_bc", bufs=1, space="PSUM") as ps_bc,
            tc.tile_pool(name="dram", bufs=1, space="DRAM") as dram,
        ):
            # ---- persistent SBUF state ----
            xT_sb = persist.tile([128, 8, ROWS], bf16)        # 64 KB/part
            wq_sb = persist.tile([128, 8, 128], bf16)
            wk_sb = persist.tile([128, 8, 128], bf16)
            wv_sb = persist.tile([128, 8, 128], bf16)
            woT_sb = persist.tile([128, 8, D], bf16)          # 16 KB/part
            qT_sb = persist.tile([128, ROWS], bf16)           # 8 KB/part
            kT_sb = persist.tile([128, ROWS], bf16)
            # v in [key-row, head, dim] layout, groups of 4 key-row tiles;
            # col HEAD is the ones column for the softmax denominator: it
            # lands on PSUM partition 64 (a legal AP base) where the DVE
            # reciprocal reads it directly -- no DMA gather (den DMAs
            # share pooled completion semaphores with the collective-gated
            # a2a_sb loads and stall the scalar queue when a peer is late)
            v2g = [persist.tile([128, 4, 2, HEAD + 1], bf16, tag=f"v2g{g}",
                                name=f"v2g{g}") for g in range(8)]
            ctx_sb = [persist.tile([64, ROWS], bf16, tag=f"ctx{h}",
                                   name=f"ctx{h}")
                      for h in range(2)]
            mask_sb = persist.tile([128, 128], bf16)
            sel_sb = persist.tile([4, 4 * HEAD], bf16)
            # row 64 = 1.0: stationary for the den-broadcast matmul,
            # partition-aligned with the denominator row (base 64)
            ones65_sb = persist.tile([65, HEAD], bf16)
            bo_sb = persist.tile([128, D], f32)
            a2a_sb = [persist.tile([128, 8, RH], bf16, tag=f"a2a{g}",
                                   name=f"a2a{g}") for g in range(4)]

            warm_sb = persist.tile([128, 512], bf16)

            # ---- HAM warmup: DVE memsets a junk tile at t=0, then a short
            #      burst of matmuls keeps the PE activity window busy so the
            #      clock gate opens (1.2 -> 2.4 GHz) before real work ----
            # enough junk to keep the PE continuously busy until the first
            # x pieces land (~12us): an idle PE drops the HAM duty to 4/8
            # right when the projections start, doubling their time
            nc.vector.memset(warm_sb[:], 0.0)
            for wi in range(0 if os.environ.get("K_NOWARM") else 16):
                ps_w = ps_work.tile([128, 512], f32, tag="work",
                                    name=f"warm{wi}")
                nc.tensor.matmul(ps_w, warm_sb[:, 0:128], warm_sb[:],
                                 start=True, stop=True)

            # ---- small loads: weights on gpsimd (wq first: it gates the
            #      very first projection matmul); tiny constants on scalar.
            #      The scalar queue carries NO bulk DMA so the engine is
            #      always free for the softmax Exp activations. ----
            nc.gpsimd.dma_start(wq_sb[:], wqT[:])
            nc.gpsimd.dma_start(wv_sb[:], wvT[:])
            nc.gpsimd.dma_start(wk_sb[:], wkT[:])
            nc.scalar.dma_start(mask_sb[:], masksq[:])
            nc.scalar.dma_start(sel_sb[:], sel[:])
            nc.scalar.dma_start(
                bo_sb[:], bass.AP(tensor=bo, offset=0,
                                  ap=[[0, 128], [1, D]]))
            nc.gpsimd.memset(ones65_sb[HEAD:HEAD + 1, :], 1.0)
            for g8 in range(8):
                nc.gpsimd.memset(v2g[g8][:, :, :, HEAD:HEAD + 1], 1.0)

            # ---- x loads. Range 0 gates the whole pipeline: its 16 fine
            #      pieces go first, byte-balanced per queue (gpsimd already
            #      carries 0.77 MB of weights so it gets one piece; sync 7,
            #      scalar 8) so all pieces land together ~19us in.
            #      Piece-major order: the cols-0:511 half that the first
            #      projections touch lands first. Ranges 1-3 follow on
            #      sync+gpsimd, 2:1 toward sync to offset woT (2 MB) on
            #      gpsimd; scalar stays free for the softmax Exps. ----
            pi = 0
            for piece in range(2):
                for kt in range(8):
                    eng = (nc.sync, nc.scalar, nc.gpsimd)[pi % 3]
                    c0 = piece * 512
                    eng.dma_start(xT_sb[:, kt, c0:c0 + 512],
                                  xT[ts(kt, 128), c0:c0 + 512])
                    pi += 1
            pi = 0
            for rr in range(1, 4):
                for kt in range(8):
                    eng = (nc.sync, nc.gpsimd)[pi % 2]
                    eng.dma_start(xT_sb[:, kt, ts(rr, 1024)],
                                  xT[ts(kt, 128), ts(rr, 1024)])
                    pi += 1
            # woT is not needed until the first output projection (~60us in);
            # keep it behind the x chunks so it can't stall the projections
            nc.gpsimd.dma_start(woT_sb[:], woT[:])

            def proj_range(rr, vbs=(0, 1)):
                """QKV projections for global rows [rr*1024, (rr+1)*1024).

                V is produced directly in [row, dim] layout (x-rows slice as
                the stationary operand, N=128). Each contraction step runs
                q, v, v, k, v, v so the two 128-column V LDWEIGHTS hide
                under the 512-wide Q/K streams."""
                for vb in vbs:
                    g8 = rr * 2 + vb
                    rt = 2 * rr + vb
                    pv = ps_work.tile([128, 4, 128], f32, tag="work",
                                      name=f"pv{g8}")
                    pq = ps_work.tile([128, 512], f32, tag="work",
                                      name=f"pq{rt}")
                    pk = ps_work.tile([128, 512], f32, tag="work",
                                      name=f"pk{rt}")
                    # V accumulation groups run one s-subtile at a time
                    # (concurrently-open groups in one PSUM bank corrupt
                    # each other); Q/K streams live in other banks.
                    vi = 0
                    for kt in range(8):
                        se = dict(start=(kt == 0), stop=(kt == 7))
                        nc.tensor.matmul(pq, wq_sb[:, kt, :],
                                         xT_sb[:, kt, ts(rt, 512)], **se)
                        for _ in range(2):
                            s, vkt = vi // 8, vi % 8
                            r0 = (g8 * 4 + s) * 128
                            nc.tensor.matmul(pv[:, s, :],
                                             xT_sb[:, vkt, r0:r0 + 128],
                                             wv_sb[:, vkt, :],
                                             start=(vkt == 0), stop=(vkt == 7))
                            vi += 1
                        nc.tensor.matmul(pk, wk_sb[:, kt, :],
                                         xT_sb[:, kt, ts(rt, 512)], **se)
                        for _ in range(2):
                            s, vkt = vi // 8, vi % 8
                            r0 = (g8 * 4 + s) * 128
                            nc.tensor.matmul(pv[:, s, :],
                                             xT_sb[:, vkt, r0:r0 + 128],
                                             wv_sb[:, vkt, :],
                                             start=(vkt == 0), stop=(vkt == 7))
                            vi += 1
                    nc.vector.tensor_copy(
                        v2g[g8][:, :, :, 0:HEAD],
                        pv.rearrange("p s (h d) -> p s h d", h=2))
                    nc.vector.tensor_copy(qT_sb[:, ts(rt, 512)], pq)
                    nc.vector.tensor_copy(kT_sb[:, ts(rt, 512)], pk)

            def attention_qt(b, qt, after_jk1=None):
                """Attention for one query tile (512 rows).

                Softmax normalization is fused per (qt, head): reciprocal of
                the PSUM denominator row, a ones-column matmul broadcasts it
                over the 64 ctx partitions, and one DVE multiply writes the
                normalized ctx straight from PSUM to SBUF. `after_jk1` is
                issued once two key blocks are in flight — the slot where a
                previous tile's norm can run without stalling on its
                reciprocal chain."""
                q0 = b * S + qt * QT
                n_k = 4 * qt + 4
                ps_av = [ps_work.tile([HEAD + 1, QT], f32, tag="work",
                                      name=f"av{b}_{qt}_{h}")
                         for h in range(2)]
                for jk in range(n_k):
                    o = jk - 4 * qt       # >=0 on the diagonal band
                    c0 = max(o, 0) * 128  # first live query column
                    k0 = b * S + jk * KT
                    ps_s = ps_scores.tile([128, 2, QT], f32, tag="sc",
                                          name=f"sc{b}_{qt}_{jk}")
                    at = attn_pool.tile([128, 2, QT], bf16,
                                        tag=f"at{jk % 2}", bufs=4,
                                        name=f"at{b}_{qt}_{jk}")
                    for h in range(2):
                        hs = slice(h * HEAD, (h + 1) * HEAD)
                        nc.tensor.matmul(
                            ps_s[:, h, c0:QT],
                            kT_sb[hs, k0:k0 + KT],
                            qT_sb[hs, q0 + c0:q0 + QT],
                            start=True, stop=True)
                    if jk < 2:
                        # pipe fill: per-head Exp halves so the first AV
                        # matmul starts ~0.5us earlier at each qt boundary
                        for h in range(2):
                            nc.scalar.activation(at[:, h, c0:QT],
                                                 ps_s[:, h, c0:QT],
                                                 Exp, scale=INV_SCALE)
                    else:
                        nc.scalar.activation(at[:, :, c0:QT],
                                             ps_s[:, :, c0:QT],
                                             Exp, scale=INV_SCALE)
                    if o >= 0:
                        # partial causal sub-block: cols [c0, c0+128)
                        nc.vector.tensor_mul(
                            at[:, :, c0:c0 + 128],
                            at[:, :, c0:c0 + 128],
                            mask_sb[:, None, :].to_broadcast([128, 2, 128]))
                    rt128 = b * 16 + jk
                    for h in range(2):
                        nc.tensor.matmul(
                            ps_av[h][:, c0:QT],
                            v2g[rt128 // 4][:, rt128 % 4, h, :],
                            at[:, h, c0:QT],
                            start=(jk == 0), stop=(jk == n_k - 1))
                    if jk == 1 and after_jk1 is not None:
                        after_jk1()
                gq = b * 4 + qt
                csts = []
                recbs = []
                for h in range(2):
                    # one copy moves ctx AND the denominator row (PSUM
                    # partition 64, the ones column) out of PSUM
                    cst = attn_pool.tile([HEAD + 1, QT], f32, tag="cst",
                                         bufs=4, name=f"cst{b}_{qt}_{h}")
                    nc.vector.tensor_copy(cst, ps_av[h][0:HEAD + 1, :])
                    # reciprocal issues eagerly on DVE so its latency
                    # hides under attention. The custom approx op (~18
                    # bits, single DVE pass, 5x cheaper) works on
                    # partition-0-based APs.
                    # reciprocal on the SCALAR engine (activation LUT),
                    # lane-aligned on partition 64: no DMA in the den
                    # chain, so no pooled-DMA-semaphore entanglement with
                    # the collective-gated a2a_sb loads
                    # (reciprocal_approx_fast is partition-0-only and the
                    # plain DVE InstReciprocal costs ~4us)
                    recb = attn_pool.tile([65, QT], bf16, tag="recb",
                                          bufs=4, name=f"recb{gq}_{h}")
                    recf = attn_pool.tile([65, QT], f32, tag="recf",
                                          bufs=4, name=f"recf{gq}_{h}")
                    # the approx op needs a base-0 AP: run it over the
                    # whole [65, QT] tile -- rows 0..63 produce junk
                    # reciprocals of ctx values that are never read; row
                    # 64 is the real denominator
                    nc.vector.reciprocal_approx_fast(recf[:], cst[:])
                    nc.vector.tensor_copy(recb[HEAD:HEAD + 1, :],
                                          recf[HEAD:HEAD + 1, :])
                    csts.append(cst)
                    recbs.append(recb)

                def norm_thunk(gq=gq, q0=q0, csts=csts, recbs=recbs):
                    # PE-side broadcast of the reciprocal row + one DVE
                    # multiply, issued later so the in-order PE stream
                    # never waits on the den chain
                    for h in range(2):
                        ps_b = ps_bc.tile([HEAD, QT], f32, tag="bc",
                                          name=f"bc{gq}_{h}")
                        nc.tensor.matmul(ps_b,
                                         ones65_sb[HEAD:HEAD + 1, :],
                                         recbs[h][HEAD:HEAD + 1, :],
                                         start=True, stop=True)
                        nc.vector.tensor_mul(
                            ctx_sb[h][:, q0:q0 + QT],
                            csts[h][0:HEAD, :],
                            ps_b[:])
                return norm_thunk

            def attention_half(b, half):
                return [attention_qt(b, 2 * half),
                        attention_qt(b, 2 * half + 1)]

            a2a_ins = [None] * 4

            def a2a_stage(b, half, qh):
                """Stage one 512-row query tile's ctx into the segment's
                A2A input buffer (issued right after that tile's norm so
                the copy hides under the next tile's attention)."""
                g = b * 2 + half
                r0 = b * S + half * (S // 2)
                if a2a_ins[g] is None:
                    a2a_ins[g] = dram.tile([8, 128, RH], bf16,
                                           tag=f"a2ain{g}", name=f"a2ain{g}")
                for h in range(2):
                    nc.sync.dma_start(
                        a2a_ins[g][qh * 4:(qh + 1) * 4,
                                   h * 64:(h + 1) * 64, :]
                        .rearrange("s p r -> p s r"),
                        ctx_sb[h][:, r0 + qh * QT:r0 + (qh + 1) * QT]
                        .rearrange("p (s r) -> p s r", s=4))

            def a2a_seg(b, half, staged=()):
                import concourse.mybir as mybir
                g = b * 2 + half
                # A2A for segment g: shard s = rows [b*2048+half*1024+s*128,+128)
                for qh in range(2):
                    if qh not in staged:
                        a2a_stage(b, half, qh)
                a2a_out = dram.tile([8, 128, RH], bf16, tag=f"a2aout{g}",
                                    name=f"a2aout{g}")
                nc.gpsimd.collective_compute(
                    "AllToAll", mybir.AluOpType.bypass,
                    replica_groups=[list(range(N_CORES))],
                    ins=[a2a_ins[g][:].opt()], outs=[a2a_out[:].opt()])
                a2a_outs[g] = a2a_out

            a2a_outs = [None] * 4

            def load_a2a(g):
                # a2a_sb load on the sync queue: the gpsimd queue head must
                # stay free for the next collective trigger
                for tb in range(2):
                    nc.sync.dma_start(
                        a2a_sb[g][:, 4 * tb:4 * tb + 4, :],
                        a2a_outs[g][4 * tb:4 * tb + 4]
                        .rearrange("t p r -> p t r"))

            def outproj_mm(g):
                # segment g rows land in out_rows[g*128:(g+1)*128]
                for nh in range(2):
                    ps = ps_work.tile([128, 512], f32, tag="work",
                                      name=f"po{g}_{nh}")
                    for t in range(8):
                        nc.tensor.matmul(ps,
                                         a2a_sb[g][:, t, :],
                                         woT_sb[:, t, ts(nh, 512)],
                                         start=(t == 0), stop=(t == 7))
                    ob = cp.tile([128, 512], f32, tag="ob", name=f"ob{g}_{nh}")
                    nc.vector.tensor_add(ob, ps, bo_sb[:, ts(nh, 512)])
                    nc.sync.dma_start(
                        out_rows[ts(g, 128), ts(nh, 512)], ob)

            def outproj(g):
                load_a2a(g)
                outproj_mm(g)

            # ---- interleaved schedule ----
            proj_range(0)
            if debug:
                nc.sync.dma_start(dbg_q[:], qT_sb[:])
                nc.sync.dma_start(dbg_k[:], kT_sb[:])
                for g8 in range(8):
                    nc.sync.dma_start(dbg_v[:, g8], v2g[g8][:])
            nt = attention_half(0, 0)
            # half of proj_range(1) runs between the attention and its
            # norms so the reciprocal chain (den DMA -> DVE recip -> bf16
            # copy) finishes under real PE work; then norms + the first A2A
            # trigger go BEFORE the rest of proj_range(1) so every core
            # stages its segment-0 ctx early and the first collective
            # (gated by the slowest core) completes well before outproj(0)
            proj_range(1, (0,))
            for t in nt:
                t()
            a2a_seg(0, 0)
            proj_range(1, (1,))
            nt = attention_half(0, 1)
            proj_range(2)
            for t in nt:
                t()
            a2a_seg(0, 1)
            nt = attention_half(1, 0)
            proj_range(3)
            for t in nt:
                t()
            a2a_seg(1, 0)
            t12 = attention_qt(1, 2)

            def norm_and_stage_12():
                t12()
                a2a_stage(1, 1, 0)  # qt=2 ctx ships under qt=3 attention

            t13 = attention_qt(1, 3, after_jk1=norm_and_stage_12)
            t13()
            a2a_seg(1, 1, staged=(0,))
            # tail: ALL a2a_sb loads live here, after the last norm — a
            # collective-gated DMA sequenced mid-pipeline (the Tile
            # scheduler hoists ready instructions regardless of issue
            # order) blocks the sync queue head AND, via pooled
            # DMA-completion semaphore baselines, the scalar den gathers,
            # serializing attention behind collectives when a peer core is
            # late. The tiny gate copies pin the loads behind the last
            # ctx write via a WAW dependency the scheduler must honor.
            for g in range(3):
                nc.gpsimd.tensor_copy(a2a_sb[g][0:1, 0:1, 0:1],
                                      ctx_sb[0][0:1, ROWS - 1:ROWS])
            load_a2a(0)
            load_a2a(1)
            load_a2a(2)
            for wi in range(48):
                ps_w = ps_scores.tile([128, 2, QT], f32, tag="sc",
                                      name=f"tailwarm{wi}")
                nc.tensor.matmul(ps_w[:, 0, :], warm_sb[:, 0:128], warm_sb[:],
                                 start=True, stop=True)
            outproj_mm(0)
            outproj_mm(1)
            outproj_mm(2)
            outproj(3)

    nc.finalize()  # Bacc.compile(): official wait-splitting & codegen passes
    return nc


def _make_masksq():
    p = np.arange(128)[:, None]
    r = np.arange(128)[None, :]
    return (p <= r).astype(BF16)


def _make_sel():
    # sel[k, u*64+m] = 1 if k == u : broadcasts den lane u over 64 partitions
    s = np.zeros((4, 4 * HEAD), np.float32)
    for u in range(4):
        s[u, u * HEAD:(u + 1) * HEAD] = 1.0
    return s.astype(BF16)


def _wlayout(wT):
    # [1024, m] -> [128, 8, m] with dst[p, t, :] = wT[t*128+p, :]
    m = wT.shape[1]
    return np.ascontiguousarray(
        wT.reshape(8, 128, m).transpose(1, 0, 2)).astype(BF16)


def _wlayout_dr8(wT):
    # [1024, m] -> fp8 [128, 4, 2, m]: dst[p, tp, j] = wT[tp*256+j*128+p, :]
    m = wT.shape[1]
    return np.ascontiguousarray(
        wT.reshape(4, 2, 128, m).transpose(2, 0, 1, 3)).astype(FP8)


def _shard_inputs(x, Wq, Wk, Wv, Wo, bo):
    xT = np.ascontiguousarray(
        x.reshape(ROWS, D).T).astype(BF16)            # [D, 4096]
    woT = _wlayout(Wo.T)                              # [128, 8, D]
    masksq = _make_masksq()
    sel = _make_sel()
    bo32 = np.ascontiguousarray(bo.astype(np.float32))
    maps = []
    for c in range(N_CORES):
        rs = slice(c * 128, (c + 1) * 128)
        maps.append({
            "xT": xT,
            "wqT": _wlayout(Wq[rs].T),
            "wkT": _wlayout(Wk[rs].T),
            "wvT": _wlayout(Wv[rs].T),
            "woT": woT,
            "bo": bo32,
            "masksq": masksq,
            "sel": sel,
        })
    return maps


def kernel(x, Wq, Wk, Wv, Wo, bo, trace=False):
    global _compiled
    _install_axon_profile_shim()
    from concourse.bass_utils import run_bass_kernel_spmd

    x = np.asarray(x, dtype=np.float32)
    Wq = np.asarray(Wq, dtype=np.float32)
    Wk = np.asarray(Wk, dtype=np.float32)
    Wv = np.asarray(Wv, dtype=np.float32)
    Wo = np.asarray(Wo, dtype=np.float32)
    bo = np.asarray(bo, dtype=np.float32)

    if _compiled is None:
        _compiled = _build_program()
    nc = _compiled

    in_maps = _shard_inputs(x, Wq, Wk, Wv, Wo, bo)
    res = run_bass_kernel_spmd(nc, in_maps, core_ids=list(range(N_CORES)),
                               trace=trace)
    out = np.empty((ROWS, D), np.float32)
    for c in range(N_CORES):
        r = res.results[c]["out_rows"]
        for g in range(4):
            b, half = g // 2, g % 2
            r0 = b * S + half * (S // 2) + c * RH
            out[r0:r0 + RH] = r[g * RH:(g + 1) * RH]
    out = out.reshape(B, S, D)
    if trace:
        kernel.last_exec_time_ns = res.exec_time_ns
        kernel.last_results = res
    return out



# revision 60
# speedup vs baseline: 1.0746x; 1.0331x over previous
"""MultiHeadAttention (B=2, S=2048, D=1024, H=16, causal) on 8 trn2 NeuronCores.

Sharding: tensor-parallel over heads (2 heads/core) for QKV projections and
attention; four AllToAlls (one per (batch, seq-half) segment) re-shard context
rows so the output projection is data-parallel over rows; bias added on
device. Host only slices/transposes/casts inputs and reassembles outputs.

Per-core output rows: global rows [c*256,(c+1)*256) (batch 0 part) and
[2048+c*256, 2048+(c+1)*256) (batch 1 part).

Schedule notes (v3):
  - x is loaded in 1024-row ranges (range 0 split into fine pieces across
    all three DMA-capable queues) so projections start ~12us in; V is
    projected directly in [row, dim] layout (x-slice stationary) so no
    transposes are needed for the attention V operand.
  - attention / norm / AllToAll / output projection are interleaved per
    segment so the middle collectives hide under attention compute and
    the PE never sees a multi-us gap (HAM duty stays high).
  - softmax denominators ride as a ones-column in the AV matmul (row 64
    of the [65, QT] PSUM tile); reciprocal_approx_fast runs over the
    whole base-0 cst tile (junk rows unread), a K=1 ones matmul
    broadcasts the reciprocal row, one DVE multiply normalizes. No DMA
    and no scalar op in this chain: DMAs here share pooled completion
    semaphores with the collective-gated a2a_sb loads and serialize
    attention behind collectives when a peer core launches late.
  - all a2a_sb loads sit at the tail behind a WAW gate copy so the Tile
    scheduler cannot hoist them (and their semaphore baselines) into the
    mid-pipeline; outproj matmuls for segments 0-2 run as real filler
    inside the final AllToAll's rendezvous window.
  - exec time = barrier skew (15-95us, environmental) + ~155us pipeline.
  - reference quirk preserved: scale = 1/(D**0.25).
"""

import os
import sys
import types

import numpy as np
import ml_dtypes

N_CORES = 8
B, S, D = 2, 2048, 1024
H = 16
HEAD = 64
ROWS = B * S               # 4096
ROWS_PER_CORE = ROWS // N_CORES  # 512
INV_SCALE = 1.0 / (D ** 0.25)
QT = 512                   # query tile (free dim)
KT = 128                   # key tile (partition dim)
RH = 128                   # rows per core per A2A segment

BF16 = ml_dtypes.bfloat16
FP8 = ml_dtypes.float8_e4m3

_compiled = None


def _install_axon_profile_shim():
    """Provide antenv.axon_hooks (missing from this image) so trace=True works,
    and neuter the artifact upload (no bucket access in-container)."""
    if "antenv.axon_hooks" not in sys.modules:
        mod = types.ModuleType("antenv.axon_hooks")
        mod._hook = None
        mod.set_axon_ntff_profile_hook = lambda h: setattr(mod, "_hook", h)
        mod.get_axon_ntff_profile_hook = lambda: mod._hook
        sys.modules["antenv.axon_hooks"] = mod
        try:
            import antenv
            antenv.axon_hooks = mod
        except ImportError:
            pass
    mod = sys.modules["antenv.axon_hooks"]
    if mod._hook is None:
        try:
            from trn_agent_boot.trn_boot import _ntff_profile_via_ctypes
            mod.set_axon_ntff_profile_hook(
                _ntff_profile_via_ctypes("/opt/axon/libaxon_pjrt.so"))
        except Exception:
            pass
    try:
        import concourse.bass_utils as bu
        bu.upload_artifacts = lambda tmpdir: tmpdir
    except Exception:
        pass


def _build_program():
    import concourse.bass as bass
    import concourse.bacc as bacc
    import concourse.mybir as mybir
    import concourse.tile as tile
    from concourse.bass import ts

    f32 = mybir.dt.float32
    f32r = mybir.dt.float32r
    bf16 = mybir.dt.bfloat16
    fp8 = mybir.dt.float8e4
    Exp = mybir.ActivationFunctionType.Exp

    nc = bacc.Bacc(num_devices=N_CORES)

    xT = nc.dram_tensor("xT", [D, ROWS], bf16, kind="ExternalInput")
    wqT = nc.dram_tensor("wqT", [128, 8, 128], bf16, kind="ExternalInput")
    wkT = nc.dram_tensor("wkT", [128, 8, 128], bf16, kind="ExternalInput")
    wvT = nc.dram_tensor("wvT", [128, 8, 128], bf16, kind="ExternalInput")
    woT = nc.dram_tensor("woT", [128, 8, D], bf16, kind="ExternalInput")
    bo = nc.dram_tensor("bo", [D], f32, kind="ExternalInput")
    masksq = nc.dram_tensor("masksq", [128, 128], bf16, kind="ExternalInput")
    sel = nc.dram_tensor("sel", [4, 4 * HEAD], bf16, kind="ExternalInput")
    out_rows = nc.dram_tensor("out_rows", [ROWS_PER_CORE, D], f32,
                              kind="ExternalOutput")
    debug = bool(os.environ.get("K_DEBUG"))
    if debug:
        dbg_q = nc.dram_tensor("dbg_q", [128, ROWS], bf16,
                               kind="ExternalOutput")
        dbg_k = nc.dram_tensor("dbg_k", [128, ROWS], bf16,
                               kind="ExternalOutput")
        dbg_v = nc.dram_tensor("dbg_v", [128, 8, 4, 2, HEAD + 1], bf16,
                               kind="ExternalOutput")

    with tile.TileContext(nc) as tc:
        with (
            tc.tile_pool(name="persist", bufs=1) as persist,
            tc.tile_pool(name="cp", bufs=4) as cp,
            tc.tile_pool(name="attn", bufs=6) as attn_pool,
            tc.tile_pool(name="ps_work", bufs=3, space="PSUM") as ps_work,
            tc.tile_pool(name="ps_scores", bufs=2, space="PSUM") as ps_scores,
            tc.tile_pool(name="ps_bc", bufs=1, space="PSUM") as ps_bc,
            tc.tile_pool(name="dram", bufs=1, space="DRAM") as dram,
        ):
            # ---- persistent SBUF state ----
            xT_sb = persist.tile([128, 8, ROWS], bf16)        # 64 KB/part
            wq_sb = persist.tile([128, 8, 128], bf16)
            wk_sb = persist.tile([128, 8, 128], bf16)
            wv_sb = persist.tile([128, 8, 128], bf16)
            woT_sb = persist.tile([128, 8, D], bf16)          # 16 KB/part
            qT_sb = persist.tile([128, ROWS], bf16)           # 8 KB/part
            kT_sb = persist.tile([128, ROWS], bf16)
            # v in [key-row, head, dim] layout, groups of 4 key-row tiles;
            # col HEAD is the ones column for the softmax denominator: it
            # lands on PSUM partition 64 (a legal AP base) where the DVE
            # reciprocal reads it directly -- no DMA gather (den DMAs
            # share pooled completion semaphores with the collective-gated
            # a2a_sb loads and stall the scalar queue when a peer is late)
            v2g = [persist.tile([128, 4, 2, HEAD + 1], bf16, tag=f"v2g{g}",
                                name=f"v2g{g}") for g in range(8)]
            ctx_sb = [persist.tile([64, ROWS], bf16, tag=f"ctx{h}",
                                   name=f"ctx{h}")
                      for h in range(2)]
            mask_sb = persist.tile([128, 128], bf16)
            sel_sb = persist.tile([4, 4 * HEAD], bf16)
            # row 64 = 1.0: stationary for the den-broadcast matmul,
            # partition-aligned with the denominator row (base 64)
            ones65_sb = persist.tile([65, HEAD], bf16)
            bo_sb = persist.tile([128, D], f32)
            a2a_sb = [persist.tile([128, 8, RH], bf16, tag=f"a2a{g}",
                                   name=f"a2a{g}") for g in range(4)]

            warm_sb = persist.tile([128, 512], bf16)

            # ---- HAM warmup: DVE memsets a junk tile at t=0, then a short
            #      burst of matmuls keeps the PE activity window busy so the
            #      clock gate opens (1.2 -> 2.4 GHz) before real work ----
            # enough junk to keep the PE continuously busy until the first
            # x pieces land (~12us): an idle PE drops the HAM duty to 4/8
            # right when the projections start, doubling their time
            nc.vector.memset(warm_sb[:], 0.0)
            for wi in range(0 if os.environ.get("K_NOWARM") else 16):
                ps_w = ps_work.tile([128, 512], f32, tag="work",
                                    name=f"warm{wi}")
                nc.tensor.matmul(ps_w, warm_sb[:, 0:128], warm_sb[:],
                                 start=True, stop=True)

            # ---- small loads: weights on gpsimd (wq first: it gates the
            #      very first projection matmul); tiny constants on scalar.
            #      The scalar queue carries NO bulk DMA so the engine is
            #      always free for the softmax Exp activations. ----
            nc.gpsimd.dma_start(wq_sb[:], wqT[:])
            nc.gpsimd.dma_start(wv_sb[:], wvT[:])
            nc.gpsimd.dma_start(wk_sb[:], wkT[:])
            nc.scalar.dma_start(mask_sb[:], masksq[:])
            nc.scalar.dma_start(sel_sb[:], sel[:])
            nc.scalar.dma_start(
                bo_sb[:], bass.AP(tensor=bo, offset=0,
                                  ap=[[0, 128], [1, D]]))
            nc.gpsimd.memset(ones65_sb[HEAD:HEAD + 1, :], 1.0)
            for g8 in range(8):
                nc.gpsimd.memset(v2g[g8][:, :, :, HEAD:HEAD + 1], 1.0)

            # ---- x loads. Range 0 gates the whole pipeline: its 16 fine
            #      pieces go first, byte-balanced per queue (gpsimd already
            #      carries 0.77 MB of weights so it gets one piece; sync 7,
            #      scalar 8) so all pieces land together ~19us in.
            #      Piece-major order: the cols-0:511 half that the first
            #      projections touch lands first. Ranges 1-3 follow on
            #      sync+gpsimd, 2:1 toward sync to offset woT (2 MB) on
            #      gpsimd; scalar stays free for the softmax Exps. ----
            pi = 0
            for piece in range(2):
                for kt in range(8):
                    eng = (nc.sync, nc.scalar, nc.gpsimd)[pi % 3]
                    c0 = piece * 512
                    eng.dma_start(xT_sb[:, kt, c0:c0 + 512],
                                  xT[ts(kt, 128), c0:c0 + 512])
                    pi += 1
            pi = 0
            for rr in range(1, 4):
                for kt in range(8):
                    eng = (nc.sync, nc.gpsimd)[pi % 2]
                    eng.dma_start(xT_sb[:, kt, ts(rr, 1024)],
                                  xT[ts(kt, 128), ts(rr, 1024)])
                    pi += 1
            # woT is not needed until the first output projection (~60us in);
            # keep it behind the x chunks so it can't stall the projections
            nc.gpsimd.dma_start(woT_sb[:], woT[:])

            def proj_range(rr, vbs=(0, 1)):
                """QKV projections for global rows [rr*1024, (rr+1)*1024).

                V is produced directly in [row, dim] layout (x-rows slice as
                the stationary operand, N=128). Each contraction step runs
                q, v, v, k, v, v so the two 128-column V LDWEIGHTS hide
                under the 512-wide Q/K streams."""
                for vb in vbs:
                    g8 = rr * 2 + vb
                    rt = 2 * rr + vb
                    pv = ps_work.tile([128, 4, 128], f32, tag="work",
                                      name=f"pv{g8}")
                    pq = ps_work.tile([128, 512], f32, tag="work",
                                      name=f"pq{rt}")
                    pk = ps_work.tile([128, 512], f32, tag="work",
                                      name=f"pk{rt}")
                    # V accumulation groups run one s-subtile at a time
                    # (concurrently-open groups in one PSUM bank corrupt
                    # each other); Q/K streams live in other banks.
                    vi = 0
                    for kt in range(8):
                        se = dict(start=(kt == 0), stop=(kt == 7))
                        nc.tensor.matmul(pq, wq_sb[:, kt, :],
                                         xT_sb[:, kt, ts(rt, 512)], **se)
                        for _ in range(2):
                            s, vkt = vi // 8, vi % 8
                            r0 = (g8 * 4 + s) * 128
                            nc.tensor.matmul(pv[:, s, :],
                                             xT_sb[:, vkt, r0:r0 + 128],
                                             wv_sb[:, vkt, :],
                                             start=(vkt == 0), stop=(vkt == 7))
                            vi += 1
                        nc.tensor.matmul(pk, wk_sb[:, kt, :],
                                         xT_sb[:, kt, ts(rt, 512)], **se)
                        for _ in range(2):
                            s, vkt = vi // 8, vi % 8
                            r0 = (g8 * 4 + s) * 128
                            nc.tensor.matmul(pv[:, s, :],
                                             xT_sb[:, vkt, r0:r0 + 128],
                                             wv_sb[:, vkt, :],
                                             start=(vkt == 0), stop=(vkt == 7))
                            vi += 1
                    nc.vector.tensor_copy(
                        v2g[g8][:, :, :, 0:HEAD],
                        pv.rearrange("p s (h d) -> p s h d", h=2))
                    nc.vector.tensor_copy(qT_sb[:, ts(rt, 512)], pq)
                    nc.vector.tensor_copy(kT_sb[:, ts(rt, 512)], pk)

            def attention_qt(b, qt, after_jk1=None):
                """Attention for one query tile (512 rows).

                Softmax normalization is fused per (qt, head): reciprocal of
                the PSUM denominator row, a ones-column matmul broadcasts it
                over the 64 ctx partitions, and one DVE multiply writes the
                normalized ctx straight from PSUM to SBUF. `after_jk1` is
                issued once two key blocks are in flight — the slot where a
                previous tile's norm can run without stalling on its
                reciprocal chain."""
                q0 = b * S + qt * QT
                n_k = 4 * qt + 4
                ps_av = [ps_work.tile([HEAD + 1, QT], f32, tag="work",
                                      name=f"av{b}_{qt}_{h}")
                         for h in range(2)]
                for jk in range(n_k):
                    o = jk - 4 * qt       # >=0 on the diagonal band
                    c0 = max(o, 0) * 128  # first live query column
                    k0 = b * S + jk * KT
                    ps_s = ps_scores.tile([128, 2, QT], f32, tag="sc",
                                          name=f"sc{b}_{qt}_{jk}")
                    at = attn_pool.tile([128, 2, QT], bf16,
                                        tag=f"at{jk % 2}", bufs=4,
                                        name=f"at{b}_{qt}_{jk}")
                    for h in range(2):
                        hs = slice(h * HEAD, (h + 1) * HEAD)
                        nc.tensor.matmul(
                            ps_s[:, h, c0:QT],
                            kT_sb[hs, k0:k0 + KT],
                            qT_sb[hs, q0 + c0:q0 + QT],
                            start=True, stop=True)
                    if jk < 2:
                        # pipe fill: per-head Exp halves so the first AV
                        # matmul starts ~0.5us earlier at each qt boundary
                        for h in range(2):
                            nc.scalar.activation(at[:, h, c0:QT],
                                                 ps_s[:, h, c0:QT],
                                                 Exp, scale=INV_SCALE)
                    else:
                        nc.scalar.activation(at[:, :, c0:QT],
                                             ps_s[:, :, c0:QT],
                                             Exp, scale=INV_SCALE)
                    if o >= 0:
                        # partial causal sub-block: cols [c0, c0+128)
                        nc.vector.tensor_mul(
                            at[:, :, c0:c0 + 128],
                            at[:, :, c0:c0 + 128],
                            mask_sb[:, None, :].to_broadcast([128, 2, 128]))
                    rt128 = b * 16 + jk
                    for h in range(2):
                        nc.tensor.matmul(
                            ps_av[h][:, c0:QT],
                            v2g[rt128 // 4][:, rt128 % 4, h, :],
                            at[:, h, c0:QT],
                            start=(jk == 0), stop=(jk == n_k - 1))
                    if jk == 1 and after_jk1 is not None:
                        after_jk1()
                gq = b * 4 + qt
                csts = []
                recbs = []
                for h in range(2):
                    # one copy moves ctx AND the denominator row (PSUM
                    # partition 64, the ones column) out of PSUM
                    cst = attn_pool.tile([HEAD + 1, QT], f32, tag="cst",
                                         bufs=4, name=f"cst{b}_{qt}_{h}")
                    nc.vector.tensor_copy(cst, ps_av[h][0:HEAD + 1, :])
                    # reciprocal issues eagerly on DVE so its latency
                    # hides under attention. The custom approx op (~18
                    # bits, single DVE pass, 5x cheaper) works on
                    # partition-0-based APs.
                    # reciprocal on the SCALAR engine (activation LUT),
                    # lane-aligned on partition 64: no DMA in the den
                    # chain, so no pooled-DMA-semaphore entanglement with
                    # the collective-gated a2a_sb loads
                    # (reciprocal_approx_fast is partition-0-only and the
                    # plain DVE InstReciprocal costs ~4us)
                    recb = attn_pool.tile([65, QT], bf16, tag="recb",
                                          bufs=4, name=f"recb{gq}_{h}")
                    recf = attn_pool.tile([65, QT], f32, tag="recf",
                                          bufs=4, name=f"recf{gq}_{h}")
                    # the approx op needs a base-0 AP: run it over the
                    # whole [65, QT] tile -- rows 0..63 produce junk
                    # reciprocals of ctx values that are never read; row
                    # 64 is the real denominator
                    nc.vector.reciprocal_approx_fast(recf[:], cst[:])
                    nc.vector.tensor_copy(recb[HEAD:HEAD + 1, :],
                                          recf[HEAD:HEAD + 1, :])
                    csts.append(cst)
                    recbs.append(recb)

                def norm_thunk(gq=gq, q0=q0, csts=csts, recbs=recbs):
                    # PE-side broadcast of the reciprocal row + one DVE
                    # multiply, issued later so the in-order PE stream
                    # never waits on the den chain
                    for h in range(2):
                        ps_b = ps_bc.tile([HEAD, QT], f32, tag="bc",
                                          name=f"bc{gq}_{h}")
                        nc.tensor.matmul(ps_b,
                                         ones65_sb[HEAD:HEAD + 1, :],
                                         recbs[h][HEAD:HEAD + 1, :],
                                         start=True, stop=True)
                        nc.vector.tensor_mul(
                            ctx_sb[h][:, q0:q0 + QT],
                            csts[h][0:HEAD, :],
                            ps_b[:])
                return norm_thunk

            def attention_half(b, half):
                return [attention_qt(b, 2 * half),
                        attention_qt(b, 2 * half + 1)]

            a2a_ins = [None] * 4

            def a2a_stage(b, half, qh):
                """Stage one 512-row query tile's ctx into the segment's
                A2A input buffer (issued right after that tile's norm so
                the copy hides under the next tile's attention)."""
                g = b * 2 + half
                r0 = b * S + half * (S // 2)
                if a2a_ins[g] is None:
                    a2a_ins[g] = dram.tile([8, 128, RH], bf16,
                                           tag=f"a2ain{g}", name=f"a2ain{g}")
                for h in range(2):
                    nc.sync.dma_start(
                        a2a_ins[g][qh * 4:(qh + 1) * 4,
                                   h * 64:(h + 1) * 64, :]
                        .rearrange("s p r -> p s r"),
                        ctx_sb[h][:, r0 + qh * QT:r0 + (qh + 1) * QT]
                        .rearrange("p (s r) -> p s r", s=4))

            def a2a_seg(b, half, staged=()):
                import concourse.mybir as mybir
                g = b * 2 + half
                # A2A for segment g: shard s = rows [b*2048+half*1024+s*128,+128)
                for qh in range(2):
                    if qh not in staged:
                        a2a_stage(b, half, qh)
                a2a_out = dram.tile([8, 128, RH], bf16, tag=f"a2aout{g}",
                                    name=f"a2aout{g}")
                nc.gpsimd.collective_compute(
                    "AllToAll", mybir.AluOpType.bypass,
                    replica_groups=[list(range(N_CORES))],
                    ins=[a2a_ins[g][:].opt()], outs=[a2a_out[:].opt()])
                a2a_outs[g] = a2a_out

            a2a_outs = [None] * 4

            def load_a2a(g):
                # a2a_sb load on the sync queue: the gpsimd queue head must
                # stay free for the next collective trigger
                for tb in range(2):
                    nc.sync.dma_start(
                        a2a_sb[g][:, 4 * tb:4 * tb + 4, :],
                        a2a_outs[g][4 * tb:4 * tb + 4]
                        .rearrange("t p r -> p t r"))

            def outproj_mm(g):
                # segment g rows land in out_rows[g*128:(g+1)*128]
                for nh in range(2):
                    ps = ps_work.tile([128, 512], f32, tag="work",
                                      name=f"po{g}_{nh}")
                    for t in range(8):
                        nc.tensor.matmul(ps,
                                         a2a_sb[g][:, t, :],
                                         woT_sb[:, t, ts(nh, 512)],
                                         start=(t == 0), stop=(t == 7))
                    ob = cp.tile([128, 512], f32, tag="ob", name=f"ob{g}_{nh}")
                    nc.vector.tensor_add(ob, ps, bo_sb[:, ts(nh, 512)])
                    nc.sync.dma_start(
                        out_rows[ts(g, 128), ts(nh, 512)], ob)

            def outproj(g):
                load_a2a(g)
                outproj_mm(g)

            # ---- interleaved schedule ----
            proj_range(0)
            if debug:
                nc.sync.dma_start(dbg_q[:], qT_sb[:])
                nc.sync.dma_start(dbg_k[:], kT_sb[:])
                for g8 in range(8):
                    nc.sync.dma_start(dbg_v[:, g8], v2g[g8][:])
            nt = attention_half(0, 0)
            # half of proj_range(1) runs between the attention and its
            # norms so the reciprocal chain (den DMA -> DVE recip -> bf16
            # copy) finishes under real PE work; then norms + the first A2A
            # trigger go BEFORE the rest of proj_range(1) so every core
            # stages its segment-0 ctx early and the first collective
            # (gated by the slowest core) completes well before outproj(0)
            proj_range(1, (0,))
            for t in nt:
                t()
            a2a_seg(0, 0)
            proj_range(1, (1,))
            nt = attention_half(0, 1)
            proj_range(2)
            for t in nt:
                t()
            a2a_seg(0, 1)
            # batch-1 segments run BIG-first: the final AllToAll's
            # rendezvous wait equals the lag the slowest core accrues over
            # the stretch since the previous rendezvous, so the short
            # segment (1,0) (12 key-blocks, ~12us) goes last instead of
            # (1,1) (28 key-blocks, ~28us)
            proj_range(3)
            t12 = attention_qt(1, 2)

            def norm_and_stage_12():
                t12()
                a2a_stage(1, 1, 0)  # qt=2 ctx ships under qt=3 attention

            t13 = attention_qt(1, 3, after_jk1=norm_and_stage_12)
            t13()
            a2a_seg(1, 1, staged=(0,))
            t10 = attention_qt(1, 0)

            def norm_and_stage_10():
                t10()
                a2a_stage(1, 0, 0)  # qt=0 ctx ships under qt=1 attention

            t11 = attention_qt(1, 1, after_jk1=norm_and_stage_10)
            t11()
            a2a_seg(1, 0, staged=(0,))
            # tail: ALL a2a_sb loads live here, after the last norm — a
            # collective-gated DMA sequenced mid-pipeline (the Tile
            # scheduler hoists ready instructions regardless of issue
            # order) blocks the sync queue head AND, via pooled
            # DMA-completion semaphore baselines, the scalar den gathers,
            # serializing attention behind collectives when a peer core is
            # late. The tiny gate copies pin the loads behind the last
            # ctx write via a WAW dependency the scheduler must honor.
            # (last norm = t11, batch-1 qt=1: cols 2560-3071)
            for g in (0, 1, 3):
                nc.gpsimd.tensor_copy(a2a_sb[g][0:1, 0:1, 0:1],
                                      ctx_sb[0][0:1, 3071:3072])
            load_a2a(0)
            load_a2a(1)
            load_a2a(3)
            for wi in range(48):
                ps_w = ps_scores.tile([128, 2, QT], f32, tag="sc",
                                      name=f"tailwarm{wi}")
                nc.tensor.matmul(ps_w[:, 0, :], warm_sb[:, 0:128], warm_sb[:],
                                 start=True, stop=True)
            outproj_mm(0)
            outproj_mm(1)
            outproj_mm(3)
            outproj(2)

    nc.finalize()  # Bacc.compile(): official wait-splitting & codegen passes
    return nc


def _make_masksq():
    p = np.arange(128)[:, None]
    r = np.arange(128)[None, :]
    return (p <= r).astype(BF16)


def _make_sel():
    # sel[k, u*64+m] = 1 if k == u : broadcasts den lane u over 64 partitions
    s = np.zeros((4, 4 * HEAD), np.float32)
    for u in range(4):
        s[u, u * HEAD:(u + 1) * HEAD] = 1.0
    return s.astype(BF16)


def _wlayout(wT):
    # [1024, m] -> [128, 8, m] with dst[p, t, :] = wT[t*128+p, :]
    m = wT.shape[1]
    return np.ascontiguousarray(
        wT.reshape(8, 128, m).transpose(1, 0, 2)).astype(BF16)


def _wlayout_dr8(wT):
    # [1024, m] -> fp8 [128, 4, 2, m]: dst[p, tp, j] = wT[tp*256+j*128+p, :]
    m = wT.shape[1]
    return np.ascontiguousarray(
        wT.reshape(4, 2, 128, m).transpose(2, 0, 1, 3)).astype(FP8)


def _shard_inputs(x, Wq, Wk, Wv, Wo, bo):
    xT = np.ascontiguousarray(
        x.reshape(ROWS, D).T).astype(BF16)            # [D, 4096]
    woT = _wlayout(Wo.T)                              # [128, 8, D]
    masksq = _make_masksq()
    sel = _make_sel()
    bo32 = np.ascontiguousarray(bo.astype(np.float32))
    maps = []
    for c in range(N_CORES):
        rs = slice(c * 128, (c + 1) * 128)
        maps.append({
            "xT": xT,
            "wqT": _wlayout(Wq[rs].T),
            "wkT": _wlayout(Wk[rs].T),
            "wvT": _wlayout(Wv[rs].T),
            "woT": woT,
            "bo": bo32,
            "masksq": masksq,
            "sel": sel,
        })
    return maps


def kernel(x, Wq, Wk, Wv, Wo, bo, trace=False):
    global _compiled
    _install_axon_profile_shim()
    from concourse.bass_utils import run_bass_kernel_spmd

    x = np.asarray(x, dtype=np.float32)
    Wq = np.asarray(Wq, dtype=np.float32)
    Wk = np.asarray(Wk, dtype=np.float32)
    Wv = np.asarray(Wv, dtype=np.float32)
    Wo = np.asarray(Wo, dtype=np.float32)
    bo = np.asarray(bo, dtype=np.float32)

    if _compiled is None:
        _compiled = _build_program()
    nc = _compiled

    in_maps = _shard_inputs(x, Wq, Wk, Wv, Wo, bo)
    res = run_bass_kernel_spmd(nc, in_maps, core_ids=list(range(N_CORES)),
                               trace=trace)
    out = np.empty((ROWS, D), np.float32)
    for c in range(N_CORES):
        r = res.results[c]["out_rows"]
        for g in range(4):
            b, half = g // 2, g % 2
            r0 = b * S + half * (S // 2) + c * RH
            out[r0:r0 + RH] = r[g * RH:(g + 1) * RH]
    out = out.reshape(B, S, D)
    if trace:
        kernel.last_exec_time_ns = res.exec_time_ns
        kernel.last_results = res
    return out

